# revision 36
# baseline (speedup 1.0000x reference)
"""TRN2 Bass kernel for nn_AttentionModuleV1 (gnn_message_passing).

Math note: the reference broadcasts features to a neighbor axis L=16 where
every slice is identical, so softmax over L is exactly uniform (1/16) and
the module collapses to (per row n of the N=16384 point axis):

    pos = relu(features  @ Wk.T)
    h   = relu(bn1(features2 @ Wv1.T))
    val = relu(bn2(h @ Wv2.T))
    vc  = sigmoid(pos @ Wv_coef.T)
    out = val + pos * vc

(xyz, Wa, Wq_coef, Wk_coef, Wqk_coef do not affect the output: they only
feed the softmax logits, which are constant along L.)

Sharding: pure data parallel over N across 8 cores (2048 rows each).

Implementation: everything bf16 (matmuls AND elementwise AND the stored
output; rel err ~3.6e-3 vs the 2e-2 gate). Host pre-swizzles all tensors
into the exact SBUF layout so each DMA moves one contiguous run per
partition; all loads share one HWDGE FIFO ring in priority order (wk,
x tile 0, biases, remaining weights, x tiles 1..3). The schedule is
software-pipelined (vc/val of tile t emitted after pos/h of tile t+1)
with per-oc drains. Engine split per 512-col tile: PE 16 matmuls
(~3.44us, the bottleneck); DVE pos drains + h(oc1) + bf16 muls + bf16
adds (~3.4us at 2x bf16 rate); ACT h(oc0) + sigmoids + val drains
(~2.9us). GpSimd is kept off the data path entirely - it measures
1.3-1.7us per [128,512] add and backlogs the tail. A warmup matmul
burst bridges preamble -> first data so the PE p-state ramp (needs ~3us
of continuous busy; any idle gap resets it to ~3x slower) is complete
when real matmuls start. The last tile stores per-oc immediately after
each add so the tail drains fast.
"""
import sys

sys.path.insert(0, "/opt/trn_rl_repo")

import numpy as np
import ml_dtypes
from concourse import bacc, mybir
import concourse.tile as tile
from concourse.bass_utils import run_bass_kernel_spmd
from concourse.alu_op_type import AluOpType

F32 = mybir.dt.float32
BF16 = mybir.dt.bfloat16
AF = mybir.ActivationFunctionType
NPBF16 = ml_dtypes.bfloat16

N_TOTAL = 16384
C = 256        # input feature channels
OUT = 256      # output channels
NCORES = 8
NSH = N_TOTAL // NCORES   # 2048 rows per core
P = 128
KC = C // P    # contraction chunks
OC = OUT // P  # output-channel chunks
NTILE = 512    # n-columns per pipeline tile (1 PSUM bank per acc)
NT = NSH // NTILE
BN_EPS = 1e-5
NWARM = 8      # warmup matmuls (~0.43us each at mid p-state)

_cache = {}


def _build():
    nc = bacc.Bacc(None, target_bir_lowering=False, debug=True)

    # host-preswizzled layouts: one contiguous run per partition
    x_d = nc.declare_dram_parameter("xsw", [NT, P, 2 * KC * NTILE], BF16,
                                    isOutput=False)
    wk_d = nc.declare_dram_parameter("wksw", [P, KC * OUT], BF16,
                                     isOutput=False)
    w1_d = nc.declare_dram_parameter("w1sw", [P, KC * OUT], BF16,
                                     isOutput=False)
    wr_d = nc.declare_dram_parameter("wrsw", [P, KC * 2 * OUT], BF16,
                                     isOutput=False)
    sb_d = nc.declare_dram_parameter("sbsw", [P, OC * 2], F32, isOutput=False)
    out_d = nc.declare_dram_parameter("osw", [NT, P, OC * NTILE], BF16,
                                      isOutput=True)

    with tile.TileContext(nc) as tc:
        with (
            tc.tile_pool(name="wpool", bufs=1) as wpool,
            tc.tile_pool(name="inpool", bufs=NT) as inpool,
            tc.tile_pool(name="midpool", bufs=2) as midpool,
            tc.tile_pool(name="outpool", bufs=2) as outpool,
            tc.tile_pool(name="psum", bufs=1, space="PSUM") as psum,
        ):
            # ---- loads in priority order on the sync HWDGE ring (FIFO)
            wk = wpool.tile([P, KC, OUT], BF16, tag="wk")
            nc.sync.dma_start(
                out=wk, in_=wk_d.ap().rearrange("p (kc o) -> p kc o", kc=KC))

            def load_x(it):
                t = inpool.tile([P, 2, KC, NTILE], BF16, tag="x")
                nc.sync.dma_start(
                    out=t,
                    in_=x_d.ap()[it].rearrange("p (s kc n) -> p s kc n",
                                               s=2, kc=KC))
                return t[:, 0], t[:, 1]

            xs = [load_x(0)]
            w1 = wpool.tile([P, KC, OUT], BF16, tag="w1")
            nc.sync.dma_start(
                out=w1, in_=w1_d.ap().rearrange("p (kc o) -> p kc o", kc=KC))
            sbt = wpool.tile([P, OC, 2], F32, tag="sbt")
            nc.sync.dma_start(
                out=sbt, in_=sb_d.ap().rearrange("p (oc c) -> p oc c", oc=OC))
            xs.append(load_x(1))
            wr = wpool.tile([P, KC, 2 * OUT], BF16, tag="wr")
            nc.sync.dma_start(
                out=wr, in_=wr_d.ap().rearrange("p (kc o) -> p kc o", kc=KC))
            for it in range(2, NT):
                xs.append(load_x(it))

            # ---- PE warmup burst (bridges preamble -> first data landing,
            # and finishes the p-state ramp before real matmuls start)
            scratch = wpool.tile([P, NTILE], BF16, tag="scratch")
            nc.gpsimd.memset(scratch, 0.0)
            wacc = psum.tile([P, NTILE], F32, tag="acc_val1")
            for _ in range(NWARM):
                nc.tensor.matmul(wacc, scratch[:, :P], scratch,
                                 start=True, stop=True)
            # dummy sigmoid: hoist the ACT table load into the DMA ramp
            dumm = wpool.tile([P, 1], F32, tag="dumm")
            nc.scalar.activation(dumm, scratch.bitcast(F32)[:, 0:1],
                                 AF.Sigmoid)

            def mm_group(w, woff, rhs, oc, tag):
                acc = psum.tile([P, NTILE], F32, tag=tag)
                for kc in range(KC):
                    nc.tensor.matmul(
                        acc,
                        w[:, kc, woff + oc * P:woff + (oc + 1) * P],
                        rhs[:, kc, :],
                        start=(kc == 0), stop=(kc == KC - 1))
                return acc

            # Software-pipelined: vc/val of tile t are emitted after pos/h
            # of tile t+1, so every drain has ~a tile of slack and the PE
            # stays dense. Per-oc granularity throughout.
            def head(it):
                x1, x2 = xs[it]
                pos = midpool.tile([P, OC, NTILE], BF16, tag="pos")
                h = midpool.tile([P, OC, NTILE], BF16, tag="h")
                # pos = relu(Wk @ x1)            (drains on DVE)
                for oc in range(OC):
                    acc = mm_group(wk, 0, x1, oc, f"acc_pos{oc}")
                    nc.vector.tensor_scalar_max(pos[:, oc, :], acc, 0.0)
                # h = relu((s1*Wv1) @ x2 + b1)   (oc0 on ACT, oc1 on DVE)
                for oc in range(OC):
                    acc = mm_group(w1, 0, x2, oc, f"acc_h{oc}")
                    if oc == 0:
                        nc.scalar.activation(h[:, oc, :], acc, AF.Relu,
                                             bias=sbt[:, oc, 0:1])
                    else:
                        nc.vector.tensor_scalar(h[:, oc, :], acc,
                                                sbt[:, oc, 0:1], 0.0,
                                                AluOpType.add, AluOpType.max)
                return it, pos, h

            def tail(state):
                it, pos, h = state
                vc = midpool.tile([P, OC, NTILE], BF16, tag="vc")
                prod = midpool.tile([P, OC, NTILE], BF16, tag="prod")
                val = midpool.tile([P, OC, NTILE], BF16, tag="val")
                outt = outpool.tile([P, OC, NTILE], BF16, tag="outt")
                # vc = sigmoid(Wvc @ pos)        (ACT)
                for oc in range(OC):
                    acc = mm_group(wr, 0, pos, oc, f"acc_vc{oc}")
                    nc.scalar.activation(vc[:, oc, :], acc, AF.Sigmoid)
                # val = relu((s2*Wv2) @ h + b2)  (ACT, bf16 out; on the
                # last tile oc0 drains on DVE so the tail chain is not
                # ACT-serial). prod/add: bf16 on DVE (2x rate).
                last = it == NT - 1
                osl = out_d.ap()[it].rearrange("p (oc n) -> p oc n", oc=OC)
                for oc in range(OC):
                    acc = mm_group(wr, OUT, h, oc, f"acc_val{oc}")
                    nc.vector.tensor_mul(prod[:, oc, :], pos[:, oc, :],
                                         vc[:, oc, :])
                    if last and oc == 0:
                        nc.vector.tensor_scalar(val[:, oc, :], acc,
                                                sbt[:, oc, 1:2], 0.0,
                                                AluOpType.add, AluOpType.max)
                    else:
                        nc.scalar.activation(val[:, oc, :], acc, AF.Relu,
                                             bias=sbt[:, oc, 1:2])
                    nc.vector.tensor_add(outt[:, oc, :], val[:, oc, :],
                                         prod[:, oc, :])
                    if last:
                        nc.sync.dma_start(out=osl[:, oc], in_=outt[:, oc])
                if not last:
                    nc.sync.dma_start(out=osl, in_=outt)

            prev = None
            for it in range(NT):
                state = head(it)
                if prev is not None:
                    tail(prev)
                prev = state
            tail(prev)
    nc.finalize()
    return nc


def _prep(inputs):
    f = np.asarray(inputs["features"], np.float32).astype(NPBF16)
    f2 = np.asarray(inputs["features2"], np.float32).astype(NPBF16)
    # xsw[core][it, p, s, kc, n] = x_s[core*NSH + it*NTILE + n, kc*P + p]
    xall = np.stack([f, f2], axis=0).reshape(
        2, NCORES, NT, NTILE, KC, P)          # s, core, it, n, kc, p
    xall = xall.transpose(1, 2, 5, 0, 4, 3)   # core, it, p, s, kc, n
    xall = np.ascontiguousarray(
        xall.reshape(NCORES, NT, P, 2 * KC * NTILE))

    eps = np.float32(BN_EPS)
    s1 = np.asarray(inputs["bn1_g"], np.float32) / np.sqrt(
        np.asarray(inputs["bn1_v"], np.float32) + eps)
    b1 = np.asarray(inputs["bn1_b"], np.float32) - np.asarray(
        inputs["bn1_m"], np.float32) * s1
    s2 = np.asarray(inputs["bn2_g"], np.float32) / np.sqrt(
        np.asarray(inputs["bn2_v"], np.float32) + eps)
    b2 = np.asarray(inputs["bn2_b"], np.float32) - np.asarray(
        inputs["bn2_m"], np.float32) * s2
    # bn scales fold into Wv1/Wv2 row scales; biases applied on-chip
    wkT = np.asarray(inputs["Wk"], np.float32).T
    wv1T = (np.asarray(inputs["Wv1"], np.float32) * s1[:, None]).T
    wv2T = (np.asarray(inputs["Wv2"], np.float32) * s2[:, None]).T
    wvcT = np.asarray(inputs["Wv_coef"], np.float32).T

    def wsw(wt):  # [C, M] -> [P, KC*M] with row p = [w(kc0) | w(kc1)]
        m = wt.shape[1]
        return wt.reshape(KC, P, m).transpose(1, 0, 2).reshape(P, KC * m)

    wksw = np.ascontiguousarray(wsw(wkT).astype(NPBF16))
    w1sw = np.ascontiguousarray(wsw(wv1T).astype(NPBF16))
    wrT = np.concatenate([wvcT, wv2T], axis=1)
    wrsw = np.ascontiguousarray(wsw(wrT).astype(NPBF16))
    # sbsw[p, oc, c]: c=0 -> b1, c=1 -> b2, channel = oc*P + p
    sbsw = np.ascontiguousarray(
        np.stack([b1, b2], axis=1).reshape(OC, P, 2)
        .transpose(1, 0, 2).reshape(P, OC * 2).astype(np.float32))

    in_maps = []
    for i in range(NCORES):
        in_maps.append({
            "xsw": xall[i],
            "wksw": wksw, "w1sw": w1sw, "wrsw": wrsw, "sbsw": sbsw,
        })
    return in_maps


def _unswizzle(osw):
    # osw [NT, P, OC*NTILE] bf16 -> [NSH, OUT] fp32
    o = np.asarray(osw).astype(np.float32).reshape(NT, P, OC, NTILE)
    return o.transpose(0, 3, 2, 1).reshape(NSH, OUT)


def _run(inputs, trace=False, trace_cores=None, tmpdir=None):
    if "nc" not in _cache:
        _cache["nc"] = _build()
    nc = _cache["nc"]
    in_maps = _prep(inputs)
    kw = {}
    if trace:
        kw = dict(trace=True, trace_cores=trace_cores or [0], tmpdir=tmpdir)
    res = run_bass_kernel_spmd(nc, in_maps, core_ids=list(range(NCORES)), **kw)
    out = np.empty((N_TOTAL, OUT), np.float32)
    for i in range(NCORES):
        out[i * NSH:(i + 1) * NSH, :] = _unswizzle(res.results[i]["osw"])
    return out, res


def kernel(**inputs):
    out, _ = _run(inputs, trace=False)
    return out


# revision 39
# speedup vs baseline: 1.0077x; 1.0077x over previous
"""TRN2 Bass kernel for nn_AttentionModuleV1 (gnn_message_passing).

Math note: the reference broadcasts features to a neighbor axis L=16 where
every slice is identical, so softmax over L is exactly uniform (1/16) and
the module collapses to (per row n of the N=16384 point axis):

    pos = relu(features  @ Wk.T)
    h   = relu(bn1(features2 @ Wv1.T))
    val = relu(bn2(h @ Wv2.T))
    vc  = sigmoid(pos @ Wv_coef.T)
    out = val + pos * vc

(xyz, Wa, Wq_coef, Wk_coef, Wqk_coef do not affect the output: they only
feed the softmax logits, which are constant along L.)

Sharding: pure data parallel over N across 8 cores (2048 rows each).

Implementation: everything bf16 (matmuls AND elementwise AND the stored
output; rel err ~3.6e-3 vs the 2e-2 gate). Host pre-swizzles all tensors
into the exact SBUF layout so each DMA moves one contiguous run per
partition; all loads share one HWDGE FIFO ring in priority order (wk,
x tile 0, biases, remaining weights, x tiles 1..3). The schedule is
software-pipelined (vc/val of tile t emitted after pos/h of tile t+1)
with per-oc drains. Engine split per 512-col tile: PE 16 matmuls
(~3.44us, the bottleneck); DVE pos drains + h(oc1) + bf16 muls + bf16
adds (~3.4us at 2x bf16 rate); ACT h(oc0) + sigmoids + val drains
(~2.9us). GpSimd is kept off the data path entirely - it measures
1.3-1.7us per [128,512] add and backlogs the tail. A warmup matmul
burst bridges preamble -> first data so the PE p-state ramp (needs ~3us
of continuous busy; any idle gap resets it to ~3x slower) is complete
when real matmuls start. The last tile stores per-oc immediately after
each add so the tail drains fast.
"""
import sys

sys.path.insert(0, "/opt/trn_rl_repo")

import numpy as np
import ml_dtypes
from concourse import bacc, mybir
import concourse.tile as tile
from concourse.bass_utils import run_bass_kernel_spmd
from concourse.alu_op_type import AluOpType

F32 = mybir.dt.float32
BF16 = mybir.dt.bfloat16
AF = mybir.ActivationFunctionType
NPBF16 = ml_dtypes.bfloat16

N_TOTAL = 16384
C = 256        # input feature channels
OUT = 256      # output channels
NCORES = 8
NSH = N_TOTAL // NCORES   # 2048 rows per core
P = 128
KC = C // P    # contraction chunks
OC = OUT // P  # output-channel chunks
NTILE = 512    # n-columns per pipeline tile (1 PSUM bank per acc)
NT = NSH // NTILE
BN_EPS = 1e-5
NWARM = 8      # warmup matmuls (~0.43us each at mid p-state)

_cache = {}


def _build():
    nc = bacc.Bacc(None, target_bir_lowering=False, debug=True)

    # host-preswizzled layouts: one contiguous run per partition
    x_d = nc.declare_dram_parameter("xsw", [NT, P, 2 * KC * NTILE], BF16,
                                    isOutput=False)
    wk_d = nc.declare_dram_parameter("wksw", [P, KC * OUT], BF16,
                                     isOutput=False)
    w1_d = nc.declare_dram_parameter("w1sw", [P, KC * OUT], BF16,
                                     isOutput=False)
    wr_d = nc.declare_dram_parameter("wrsw", [P, KC * 2 * OUT], BF16,
                                     isOutput=False)
    sb_d = nc.declare_dram_parameter("sbsw", [P, OC * 2], F32, isOutput=False)
    out_d = nc.declare_dram_parameter("osw", [NT, P, OC * NTILE], BF16,
                                      isOutput=True)

    with tile.TileContext(nc) as tc:
        with (
            tc.tile_pool(name="wpool", bufs=1) as wpool,
            tc.tile_pool(name="inpool", bufs=NT) as inpool,
            tc.tile_pool(name="midpool", bufs=2) as midpool,
            tc.tile_pool(name="outpool", bufs=2) as outpool,
            tc.tile_pool(name="psum", bufs=1, space="PSUM") as psum,
        ):
            # ---- loads in priority order on the sync HWDGE ring (FIFO)
            wk = wpool.tile([P, KC, OUT], BF16, tag="wk")
            nc.sync.dma_start(
                out=wk, in_=wk_d.ap().rearrange("p (kc o) -> p kc o", kc=KC))

            def load_x(it):
                t = inpool.tile([P, 2, KC, NTILE], BF16, tag="x")
                nc.sync.dma_start(
                    out=t,
                    in_=x_d.ap()[it].rearrange("p (s kc n) -> p s kc n",
                                               s=2, kc=KC))
                return t[:, 0], t[:, 1]

            xs = [load_x(0)]
            w1 = wpool.tile([P, KC, OUT], BF16, tag="w1")
            nc.sync.dma_start(
                out=w1, in_=w1_d.ap().rearrange("p (kc o) -> p kc o", kc=KC))
            sbt = wpool.tile([P, OC, 2], F32, tag="sbt")
            nc.sync.dma_start(
                out=sbt, in_=sb_d.ap().rearrange("p (oc c) -> p oc c", oc=OC))
            xs.append(load_x(1))
            wr = wpool.tile([P, KC, 2 * OUT], BF16, tag="wr")
            nc.sync.dma_start(
                out=wr, in_=wr_d.ap().rearrange("p (kc o) -> p kc o", kc=KC))
            for it in range(2, NT):
                xs.append(load_x(it))

            # ---- PE warmup burst (bridges preamble -> first data landing,
            # and finishes the p-state ramp before real matmuls start)
            scratch = wpool.tile([P, NTILE], BF16, tag="scratch")
            nc.gpsimd.memset(scratch, 0.0)
            wacc = psum.tile([P, NTILE], F32, tag="acc_val1")
            for _ in range(NWARM):
                nc.tensor.matmul(wacc, scratch[:, :P], scratch,
                                 start=True, stop=True)
            # two short warmups: finer-grained handoff to the first real
            # matmul (a 512-row warmup quantizes the idle gap at ~0.43us)
            for _ in range(2):
                nc.tensor.matmul(wacc[:, :P], scratch[:, :P],
                                 scratch[:, :P], start=True, stop=True)
            # dummy sigmoid: hoist the ACT table load into the DMA ramp
            dumm = wpool.tile([P, 1], F32, tag="dumm")
            nc.scalar.activation(dumm, scratch.bitcast(F32)[:, 0:1],
                                 AF.Sigmoid)

            def mm_group(w, woff, rhs, oc, tag):
                acc = psum.tile([P, NTILE], F32, tag=tag)
                for kc in range(KC):
                    nc.tensor.matmul(
                        acc,
                        w[:, kc, woff + oc * P:woff + (oc + 1) * P],
                        rhs[:, kc, :],
                        start=(kc == 0), stop=(kc == KC - 1))
                return acc

            # Software-pipelined: vc/val of tile t are emitted after pos/h
            # of tile t+1, so every drain has ~a tile of slack and the PE
            # stays dense. Per-oc granularity throughout.
            def head(it):
                x1, x2 = xs[it]
                pos = midpool.tile([P, OC, NTILE], BF16, tag="pos")
                h = midpool.tile([P, OC, NTILE], BF16, tag="h")
                # pos = relu(Wk @ x1)            (drains on DVE)
                for oc in range(OC):
                    acc = mm_group(wk, 0, x1, oc, f"acc_pos{oc}")
                    nc.vector.tensor_scalar_max(pos[:, oc, :], acc, 0.0)
                # h = relu((s1*Wv1) @ x2 + b1)   (oc0 on ACT, oc1 on DVE)
                for oc in range(OC):
                    acc = mm_group(w1, 0, x2, oc, f"acc_h{oc}")
                    if oc == 0:
                        nc.scalar.activation(h[:, oc, :], acc, AF.Relu,
                                             bias=sbt[:, oc, 0:1])
                    else:
                        nc.vector.tensor_scalar(h[:, oc, :], acc,
                                                sbt[:, oc, 0:1], 0.0,
                                                AluOpType.add, AluOpType.max)
                return it, pos, h

            def tail(state):
                it, pos, h = state
                vc = midpool.tile([P, OC, NTILE], BF16, tag="vc")
                prod = midpool.tile([P, OC, NTILE], BF16, tag="prod")
                val = midpool.tile([P, OC, NTILE], BF16, tag="val")
                outt = outpool.tile([P, OC, NTILE], BF16, tag="outt")
                # vc = sigmoid(Wvc @ pos)        (ACT)
                for oc in range(OC):
                    acc = mm_group(wr, 0, pos, oc, f"acc_vc{oc}")
                    nc.scalar.activation(vc[:, oc, :], acc, AF.Sigmoid)
                # val = relu((s2*Wv2) @ h + b2)  (ACT, bf16 out; on the
                # last tile oc0 drains on DVE so the tail chain is not
                # ACT-serial). prod/add: bf16 on DVE (2x rate).
                last = it == NT - 1
                osl = out_d.ap()[it].rearrange("p (oc n) -> p oc n", oc=OC)
                for oc in range(OC):
                    acc = mm_group(wr, OUT, h, oc, f"acc_val{oc}")
                    nc.vector.tensor_mul(prod[:, oc, :], pos[:, oc, :],
                                         vc[:, oc, :])
                    if last and oc == 0:
                        nc.vector.tensor_scalar(val[:, oc, :], acc,
                                                sbt[:, oc, 1:2], 0.0,
                                                AluOpType.add, AluOpType.max)
                    else:
                        nc.scalar.activation(val[:, oc, :], acc, AF.Relu,
                                             bias=sbt[:, oc, 1:2])
                    nc.vector.tensor_add(outt[:, oc, :], val[:, oc, :],
                                         prod[:, oc, :])
                    if last:
                        nc.sync.dma_start(out=osl[:, oc], in_=outt[:, oc])
                if not last:
                    nc.sync.dma_start(out=osl, in_=outt)

            prev = None
            for it in range(NT):
                state = head(it)
                if prev is not None:
                    tail(prev)
                prev = state
            tail(prev)
    nc.finalize()
    return nc


def _prep(inputs):
    f = np.asarray(inputs["features"], np.float32).astype(NPBF16)
    f2 = np.asarray(inputs["features2"], np.float32).astype(NPBF16)
    # xsw[core][it, p, s, kc, n] = x_s[core*NSH + it*NTILE + n, kc*P + p]
    xall = np.stack([f, f2], axis=0).reshape(
        2, NCORES, NT, NTILE, KC, P)          # s, core, it, n, kc, p
    xall = xall.transpose(1, 2, 5, 0, 4, 3)   # core, it, p, s, kc, n
    xall = np.ascontiguousarray(
        xall.reshape(NCORES, NT, P, 2 * KC * NTILE))

    eps = np.float32(BN_EPS)
    s1 = np.asarray(inputs["bn1_g"], np.float32) / np.sqrt(
        np.asarray(inputs["bn1_v"], np.float32) + eps)
    b1 = np.asarray(inputs["bn1_b"], np.float32) - np.asarray(
        inputs["bn1_m"], np.float32) * s1
    s2 = np.asarray(inputs["bn2_g"], np.float32) / np.sqrt(
        np.asarray(inputs["bn2_v"], np.float32) + eps)
    b2 = np.asarray(inputs["bn2_b"], np.float32) - np.asarray(
        inputs["bn2_m"], np.float32) * s2
    # bn scales fold into Wv1/Wv2 row scales; biases applied on-chip
    wkT = np.asarray(inputs["Wk"], np.float32).T
    wv1T = (np.asarray(inputs["Wv1"], np.float32) * s1[:, None]).T
    wv2T = (np.asarray(inputs["Wv2"], np.float32) * s2[:, None]).T
    wvcT = np.asarray(inputs["Wv_coef"], np.float32).T

    def wsw(wt):  # [C, M] -> [P, KC*M] with row p = [w(kc0) | w(kc1)]
        m = wt.shape[1]
        return wt.reshape(KC, P, m).transpose(1, 0, 2).reshape(P, KC * m)

    wksw = np.ascontiguousarray(wsw(wkT).astype(NPBF16))
    w1sw = np.ascontiguousarray(wsw(wv1T).astype(NPBF16))
    wrT = np.concatenate([wvcT, wv2T], axis=1)
    wrsw = np.ascontiguousarray(wsw(wrT).astype(NPBF16))
    # sbsw[p, oc, c]: c=0 -> b1, c=1 -> b2, channel = oc*P + p
    sbsw = np.ascontiguousarray(
        np.stack([b1, b2], axis=1).reshape(OC, P, 2)
        .transpose(1, 0, 2).reshape(P, OC * 2).astype(np.float32))

    in_maps = []
    for i in range(NCORES):
        in_maps.append({
            "xsw": xall[i],
            "wksw": wksw, "w1sw": w1sw, "wrsw": wrsw, "sbsw": sbsw,
        })
    return in_maps


def _unswizzle(osw):
    # osw [NT, P, OC*NTILE] bf16 -> [NSH, OUT] fp32
    o = np.asarray(osw).astype(np.float32).reshape(NT, P, OC, NTILE)
    return o.transpose(0, 3, 2, 1).reshape(NSH, OUT)


def _run(inputs, trace=False, trace_cores=None, tmpdir=None):
    if "nc" not in _cache:
        _cache["nc"] = _build()
    nc = _cache["nc"]
    in_maps = _prep(inputs)
    kw = {}
    if trace:
        kw = dict(trace=True, trace_cores=trace_cores or [0], tmpdir=tmpdir)
    res = run_bass_kernel_spmd(nc, in_maps, core_ids=list(range(NCORES)), **kw)
    out = np.empty((N_TOTAL, OUT), np.float32)
    for i in range(NCORES):
        out[i * NSH:(i + 1) * NSH, :] = _unswizzle(res.results[i]["osw"])
    return out, res


def kernel(**inputs):
    out, _ = _run(inputs, trace=False)
    return out
